# revision 5
# baseline (speedup 1.0000x reference)
"""MoE router kernel for Trainium2 (Bass/Tile), SPMD over 8 NeuronCores.

Reference computation (full problem):
    logits = einsum('bsd,ed->bse', x, W)     x: [4, 4096, 2048] f32, W: [8, 2048] f32
    top_vals, top_idx = top_k(logits, 2)
    gates = softmax(top_vals, axis=-1)
    returns (top_idx int32, gates f32, logits f32)

Sharding: data parallel over tokens (batch*seq = 16384) -> 2048 tokens/core,
router weights replicated. Each core:
  - token t = 16*p + b  (p = SBUF partition 0..127, b = token block 0..15), so
    every DMA is contiguous per partition (x rows: 8KB, outputs: 128-512B).
  - Per super-block of 512 tokens: PE-transpose x 128x128 chunks into PSUM,
    copy to SBUF (ACT/DVE alternating), then router matmul with W^T chunks
    stationary ([128,8]) and x^T moving (N=512), accumulating logits^T [8,512]
    in PSUM over the 16 contraction chunks.
  - PE-transpose logits^T -> [128 tok, 8]; DVE max/max_index give the top-8
    sorted values + indices per token (exactly top-k for E=8); batched
    exp/sum/reciprocal softmax over the top-2.
"""

import os
import sys

for _p in ("/opt/trn_rl_repo", "/root/.axon_site", "/root/.axon_site/_ro/trn_rl_repo",
           "/root/.axon_site/_ro/pypackages"):
    if os.path.isdir(_p) and _p not in sys.path:
        sys.path.append(_p)

import numpy as np

import concourse.bass as bass  # noqa: F401
import concourse.mybir as mybir
from concourse import bacc, tile
from concourse.bass_utils import run_bass_kernel_spmd
from concourse.masks import make_identity

# ---- problem constants (hardcoded; kernel.py must be self-contained) ----
B, S, D, E, TOPK = 4, 4096, 2048, 8, 2
N_CORES = 8
T_ALL = B * S                 # 16384 tokens
T_LOC = T_ALL // N_CORES      # 2048 tokens per core
P = 128                       # SBUF partitions
NB = T_LOC // P               # 16 token blocks per core
SBK = 4                       # blocks per super-block (512 tokens)
NSB = NB // SBK               # 4 super-blocks
DC = D // P                   # 16 contraction chunks

f32 = mybir.dt.float32
f32r = mybir.dt.float32r
i32 = mybir.dt.int32
u32 = mybir.dt.uint32

def build_nc():
    nc = bacc.Bacc(
        "TRN2",
        target_bir_lowering=False,
        debug=False,
        enable_asserts=False,
        num_devices=N_CORES,
    )
    x = nc.dram_tensor("x", [T_LOC, D], f32, kind="ExternalInput").ap()
    w = nc.dram_tensor("w", [E, D], f32, kind="ExternalInput").ap()
    idx_d = nc.dram_tensor("indices", [T_LOC, TOPK], i32, kind="ExternalOutput").ap()
    gat_d = nc.dram_tensor("gates", [T_LOC, TOPK], f32, kind="ExternalOutput").ap()
    log_d = nc.dram_tensor("logits", [T_LOC, E], f32, kind="ExternalOutput").ap()

    # token t = 16*p + b
    x_v = x.rearrange("(p b) d -> b p d", b=NB)           # [16, 128, 2048]
    idx_v = idx_d.rearrange("(p b) k -> p (b k)", b=NB)   # [128, 32]
    gat_v = gat_d.rearrange("(p b) k -> p (b k)", b=NB)   # [128, 32]
    log_v = log_d.rearrange("(p b) e -> p (b e)", b=NB)   # [128, 128]

    with tile.TileContext(nc) as tc:
        with (
            tc.tile_pool(name="const", bufs=1) as cpool,
            tc.tile_pool(name="xin", bufs=2 * SBK) as xpool,
            tc.tile_pool(name="xt", bufs=4) as xtpool,
            tc.tile_pool(name="acc", bufs=1) as apool,
            tc.tile_pool(name="ps_xt", bufs=3, space="PSUM") as pxt,
            tc.tile_pool(name="ps_out", bufs=4, space="PSUM") as pout,
            tc.tile_pool(name="ps_tr", bufs=1, space="PSUM") as ptr,
        ):
            # ---------------- constants ----------------
            ident = cpool.tile([P, P], f32)
            make_identity(nc, ident[:])
            w_sb = cpool.tile([E, D], f32)
            nc.sync.dma_start(out=w_sb[:], in_=w)
            # W^T chunks: wt_sb[:, 8c:8c+8] = W[:, 128c:128c+128].T  ([128 d, 8 e])
            wt_sb = cpool.tile([P, DC * E], f32)
            for h in range(2):
                wt_ps = ptr.tile([P, P], f32, tag="tr")
                for ci in range(8):
                    c = 8 * h + ci
                    nc.tensor.transpose(
                        wt_ps[:, E * ci : E * (ci + 1)],
                        w_sb[:, P * c : P * (c + 1)],
                        ident[0:E, 0:E],
                    )
                nc.vector.tensor_copy(wt_sb[:, 64 * h : 64 * (h + 1)], wt_ps[:, 0:64])

            # HAM warmup: ~4us of back-to-back matmuls so the PE clock-gate
            # opens to 8/8 before the transpose/matmul stream begins.
            warm_ps = ptr.tile([P, P], f32, tag="tr")
            for _ in range(10):
                nc.tensor.matmul(warm_ps[:], ident[:], ident[:], start=True, stop=True)

            # ---------------- per-core accumulators ----------------
            out_log3 = apool.tile([P, NB, E], f32)    # logits, token-major layout
            tops3 = apool.tile([P, NB, E], f32)       # top-8 sorted values
            idx3 = apool.tile([P, NB, E], u32)        # their indices
            out_idx3 = apool.tile([P, NB, TOPK], i32)
            out_gat3 = apool.tile([P, NB, TOPK], f32)
            ex_in = apool.tile([P, NB, TOPK], f32)
            ex = apool.tile([P, NB, TOPK], f32)
            ssum = apool.tile([P, NB, 1], f32)
            rcp = apool.tile([P, NB, 1], f32)

            # ---------------- main loop over super-blocks ----------------
            for s in range(NSB):
                xbs = []
                for j in range(SBK):
                    b = SBK * s + j
                    xb = xpool.tile([P, D], f32, tag="xb")
                    nc.sync.dma_start(out=xb[:], in_=x_v[b])
                    xbs.append(xb)

                # per-block logits accumulators [128 tok, 8]
                out_ps = [
                    pout.tile([P, E], f32, tag="out_ps", name=f"out_ps_{s}_{j}")
                    for j in range(SBK)
                ]

                for c in range(DC):
                    # x^T chunk c for all 4 blocks: xt[:, 128j + q] = x[tok q of
                    # block 4s+j, 128c + p]
                    xt_ps = pxt.tile([P, 4 * P], f32, tag="xt_ps")
                    for j in range(SBK):
                        nc.tensor.transpose(
                            xt_ps[:, P * j : P * (j + 1)],
                            xbs[j][:, P * c : P * (c + 1)],
                            ident[:],
                        )
                    xt = xtpool.tile([P, 4 * P], f32, tag="xt")
                    eng = nc.scalar if c % 2 == 0 else nc.vector
                    if eng is nc.scalar:
                        eng.copy(xt[:], xt_ps[:])
                    else:
                        eng.tensor_copy(xt[:], xt_ps[:])
                    # router matmul: stationary = x^T chunk (128 cols), moving =
                    # W^T chunk [128, 8] (8 fp32 rows -> ~floor-cost matmuls)
                    for j in range(SBK):
                        nc.tensor.matmul(
                            out_ps[j][:],
                            xt[:, P * j : P * (j + 1)],
                            wt_sb[:, E * c : E * (c + 1)],
                            start=(c == 0),
                            stop=(c == DC - 1),
                        )

                # logits land token-major; copy out + top-k per block
                for j in range(SBK):
                    b = SBK * s + j
                    nc.vector.tensor_copy(out_log3[:, b, :], out_ps[j][:])
                    nc.vector.max(out=tops3[:, b, :], in_=out_log3[:, b, :])
                    nc.vector.max_index(
                        out=idx3[:, b, :],
                        in_max=tops3[:, b, :],
                        in_values=out_log3[:, b, :],
                    )

            # ---------------- batched softmax over top-2 ----------------
            nc.vector.tensor_sub(
                ex_in[:],
                tops3[:, :, 0:TOPK],
                tops3[:, :, 0:1].to_broadcast([P, NB, TOPK]),
            )
            nc.scalar.activation(ex[:], ex_in[:], mybir.ActivationFunctionType.Exp)
            nc.vector.tensor_reduce(
                ssum[:], ex[:], axis=mybir.AxisListType.X, op=mybir.AluOpType.add
            )
            nc.vector.reciprocal(rcp[:], ssum[:])
            nc.vector.tensor_mul(
                out_gat3[:], ex[:], rcp.to_broadcast([P, NB, TOPK])
            )
            nc.vector.tensor_copy(out_idx3[:], idx3[:, :, 0:TOPK])

            # ---------------- outputs ----------------
            nc.sync.dma_start(out=idx_v, in_=out_idx3[:])
            nc.sync.dma_start(out=gat_v, in_=out_gat3[:])
            nc.sync.dma_start(out=log_v, in_=out_log3[:])

    nc.compile()
    return nc


_NC_CACHE = None


def _get_nc():
    global _NC_CACHE
    if _NC_CACHE is None:
        _NC_CACHE = build_nc()
    return _NC_CACHE


def _shard_inputs(x: np.ndarray, router_weights: np.ndarray):
    xf = np.ascontiguousarray(np.asarray(x, dtype=np.float32).reshape(T_ALL, D))
    wf = np.ascontiguousarray(np.asarray(router_weights, dtype=np.float32))
    in_maps = []
    for i in range(N_CORES):
        in_maps.append(
            {
                "x": xf[i * T_LOC : (i + 1) * T_LOC],
                "w": wf,
            }
        )
    return in_maps


def _assemble(results):
    idx = np.concatenate([r["indices"] for r in results], axis=0).reshape(B, S, TOPK)
    gat = np.concatenate([r["gates"] for r in results], axis=0).reshape(B, S, TOPK)
    lgt = np.concatenate([r["logits"] for r in results], axis=0).reshape(B, S, E)
    return idx.astype(np.int32), gat.astype(np.float32), lgt.astype(np.float32)


def kernel(x: np.ndarray, router_weights: np.ndarray):
    nc = _get_nc()
    in_maps = _shard_inputs(x, router_weights)
    res = run_bass_kernel_spmd(nc, in_maps, core_ids=list(range(N_CORES)))
    return _assemble(res.results)


def kernel_traced(x: np.ndarray, router_weights: np.ndarray, trace_cores=None):
    """Like kernel() but profiles; returns (outputs, BassKernelResults)."""
    nc = _get_nc()
    in_maps = _shard_inputs(x, router_weights)
    res = run_bass_kernel_spmd(
        nc,
        in_maps,
        core_ids=list(range(N_CORES)),
        trace=True,
        trace_cores=trace_cores or [0],
    )
    return _assemble(res.results), res


# revision 7
# speedup vs baseline: 1.4826x; 1.4826x over previous
"""MoE router kernel for Trainium2 (Bass/Tile), SPMD over 8 NeuronCores.

Reference computation (full problem):
    logits = einsum('bsd,ed->bse', x, W)     x: [4, 4096, 2048] f32, W: [8, 2048] f32
    top_vals, top_idx = top_k(logits, 2)
    gates = softmax(top_vals, axis=-1)
    returns (top_idx int32, gates f32, logits f32)

Sharding: data parallel over tokens (batch*seq = 16384) -> 2048 tokens/core,
router weights replicated. Each core:
  - token t = 16*p + b  (p = SBUF partition 0..127, b = token block 0..15), so
    every DMA is contiguous per partition (x rows: 8KB, outputs: 128-512B).
  - Per super-block of 512 tokens: PE-transpose x 128x128 chunks into PSUM,
    copy to SBUF (ACT/DVE alternating), then router matmul with W^T chunks
    stationary ([128,8]) and x^T moving (N=512), accumulating logits^T [8,512]
    in PSUM over the 16 contraction chunks.
  - PE-transpose logits^T -> [128 tok, 8]; DVE max/max_index give the top-8
    sorted values + indices per token (exactly top-k for E=8); batched
    exp/sum/reciprocal softmax over the top-2.
"""

import os
import sys

for _p in ("/opt/trn_rl_repo", "/root/.axon_site", "/root/.axon_site/_ro/trn_rl_repo",
           "/root/.axon_site/_ro/pypackages"):
    if os.path.isdir(_p) and _p not in sys.path:
        sys.path.append(_p)

import numpy as np

import concourse.bass as bass  # noqa: F401
import concourse.mybir as mybir
from concourse import bacc, tile
from concourse.bass_utils import run_bass_kernel_spmd
from concourse.masks import make_identity

# ---- problem constants (hardcoded; kernel.py must be self-contained) ----
B, S, D, E, TOPK = 4, 4096, 2048, 8, 2
N_CORES = 8
T_ALL = B * S                 # 16384 tokens
T_LOC = T_ALL // N_CORES      # 2048 tokens per core
P = 128                       # SBUF partitions
NB = T_LOC // P               # 16 token blocks per core
SBK = 4                       # blocks per super-block (512 tokens)
NSB = NB // SBK               # 4 super-blocks
DC = D // P                   # 16 contraction chunks

f32 = mybir.dt.float32
f32r = mybir.dt.float32r
i32 = mybir.dt.int32
u32 = mybir.dt.uint32

def build_nc():
    nc = bacc.Bacc(
        "TRN2",
        target_bir_lowering=False,
        debug=False,
        enable_asserts=False,
        num_devices=N_CORES,
    )
    x = nc.dram_tensor("x", [T_LOC, D], f32, kind="ExternalInput").ap()
    w = nc.dram_tensor("w", [E, D], f32, kind="ExternalInput").ap()
    idx_d = nc.dram_tensor("indices", [T_LOC, TOPK], i32, kind="ExternalOutput").ap()
    gat_d = nc.dram_tensor("gates", [T_LOC, TOPK], f32, kind="ExternalOutput").ap()
    log_d = nc.dram_tensor("logits", [T_LOC, E], f32, kind="ExternalOutput").ap()

    # token t = 16*p + b
    x_v = x.rearrange("(p b) d -> b p d", b=NB)           # [16, 128, 2048]
    idx_v = idx_d.rearrange("(p b) k -> p (b k)", b=NB)   # [128, 32]
    gat_v = gat_d.rearrange("(p b) k -> p (b k)", b=NB)   # [128, 32]
    log_v = log_d.rearrange("(p b) e -> p (b e)", b=NB)   # [128, 128]

    with tile.TileContext(nc) as tc:
        with (
            tc.tile_pool(name="const", bufs=1) as cpool,
            tc.tile_pool(name="xin", bufs=2 * SBK) as xpool,
            tc.tile_pool(name="xt", bufs=4) as xtpool,
            tc.tile_pool(name="acc", bufs=1) as apool,
            tc.tile_pool(name="ps_xt", bufs=3, space="PSUM") as pxt,
            tc.tile_pool(name="ps_out", bufs=2, space="PSUM") as pout,
            tc.tile_pool(name="ps_tr", bufs=2, space="PSUM") as ptr,
        ):
            # ---------------- constants ----------------
            ident = cpool.tile([P, P], f32)
            make_identity(nc, ident[:])
            w_sb = cpool.tile([E, D], f32)
            nc.sync.dma_start(out=w_sb[:], in_=w)
            # W^T chunks: wt_sb[:, 8c:8c+8] = W[:, 128c:128c+128].T  ([128 d, 8 e])
            wt_sb = cpool.tile([P, DC * E], f32)
            for h in range(2):
                wt_ps = ptr.tile([P, P], f32, tag="tr")
                for ci in range(8):
                    c = 8 * h + ci
                    nc.tensor.transpose(
                        wt_ps[:, E * ci : E * (ci + 1)],
                        w_sb[:, P * c : P * (c + 1)],
                        ident[0:E, 0:E],
                    )
                nc.vector.tensor_copy(wt_sb[:, 64 * h : 64 * (h + 1)], wt_ps[:, 0:64])

            # HAM warmup: ~4us of back-to-back matmuls so the PE clock-gate
            # opens to 8/8 before the transpose/matmul stream begins.
            warm_ps = ptr.tile([P, P], f32, tag="tr")
            for _ in range(10):
                nc.tensor.matmul(warm_ps[:], ident[:], ident[:], start=True, stop=True)

            # ---------------- per-core accumulators ----------------
            out_log3 = apool.tile([P, NB, E], f32)    # logits, token-major layout
            tops3 = apool.tile([P, NB, E], f32)       # top-8 sorted values
            idx3 = apool.tile([P, NB, E], u32)        # their indices
            out_idx3 = apool.tile([P, NB, TOPK], i32)
            out_gat3 = apool.tile([P, NB, TOPK], f32)
            ex_in = apool.tile([P, NB, TOPK], f32)
            ex = apool.tile([P, NB, TOPK], f32)
            ssum = apool.tile([P, NB, 1], f32)
            rcp = apool.tile([P, NB, 1], f32)

            # ---------------- main loop over super-blocks ----------------
            for s in range(NSB):
                xbs = []
                for j in range(SBK):
                    b = SBK * s + j
                    xb = xpool.tile([P, D], f32, tag="xb")
                    nc.sync.dma_start(out=xb[:], in_=x_v[b])
                    xbs.append(xb)

                # logits^T [8, 512] accumulated over the 16 contraction chunks
                logT_ps = pout.tile([E, 4 * P], f32, tag="lgT")
                for c in range(DC):
                    # x^T chunk c for all 4 blocks: xt[:, 128j + q] = x[tok q of
                    # block 4s+j, 128c + p]
                    xt_ps = pxt.tile([P, 4 * P], f32, tag="xt_ps")
                    for j in range(SBK):
                        nc.tensor.transpose(
                            xt_ps[:, P * j : P * (j + 1)],
                            xbs[j][:, P * c : P * (c + 1)],
                            ident[:],
                        )
                    xt = xtpool.tile([P, 4 * P], f32, tag="xt")
                    eng = nc.scalar if c % 2 == 0 else nc.vector
                    if eng is nc.scalar:
                        eng.copy(xt[:], xt_ps[:])
                    else:
                        eng.tensor_copy(xt[:], xt_ps[:])
                    # router matmul: stationary = W^T chunk [128, 8] (cheap
                    # 8-col weight load), moving = x^T chunk, N=512
                    nc.tensor.matmul(
                        logT_ps[:],
                        wt_sb[:, E * c : E * (c + 1)],
                        xt[:],
                        start=(c == 0),
                        stop=(c == DC - 1),
                    )

                logT_sb = xtpool.tile([E, 4 * P], f32, tag="lgTsb")
                nc.scalar.copy(logT_sb[:], logT_ps[:])

                # transpose logits^T -> [128 tok, 8] per block; top-k
                for j in range(SBK):
                    b = SBK * s + j
                    ltr_ps = ptr.tile([P, E], f32, tag="tr")
                    nc.tensor.transpose(
                        ltr_ps[:], logT_sb[:, P * j : P * (j + 1)], ident[0:E, 0:E]
                    )
                    nc.vector.tensor_copy(out_log3[:, b, :], ltr_ps[:])
                    nc.vector.max(out=tops3[:, b, :], in_=out_log3[:, b, :])
                    nc.vector.max_index(
                        out=idx3[:, b, :],
                        in_max=tops3[:, b, :],
                        in_values=out_log3[:, b, :],
                    )

            # ---------------- batched softmax over top-2 ----------------
            nc.vector.tensor_sub(
                ex_in[:],
                tops3[:, :, 0:TOPK],
                tops3[:, :, 0:1].to_broadcast([P, NB, TOPK]),
            )
            nc.scalar.activation(ex[:], ex_in[:], mybir.ActivationFunctionType.Exp)
            nc.vector.tensor_reduce(
                ssum[:], ex[:], axis=mybir.AxisListType.X, op=mybir.AluOpType.add
            )
            nc.vector.reciprocal(rcp[:], ssum[:])
            nc.vector.tensor_mul(
                out_gat3[:], ex[:], rcp.to_broadcast([P, NB, TOPK])
            )
            nc.vector.tensor_copy(out_idx3[:], idx3[:, :, 0:TOPK])

            # ---------------- outputs ----------------
            nc.sync.dma_start(out=idx_v, in_=out_idx3[:])
            nc.sync.dma_start(out=gat_v, in_=out_gat3[:])
            nc.sync.dma_start(out=log_v, in_=out_log3[:])

    nc.compile()
    return nc


_NC_CACHE = None


def _get_nc():
    global _NC_CACHE
    if _NC_CACHE is None:
        _NC_CACHE = build_nc()
    return _NC_CACHE


def _shard_inputs(x: np.ndarray, router_weights: np.ndarray):
    xf = np.ascontiguousarray(np.asarray(x, dtype=np.float32).reshape(T_ALL, D))
    wf = np.ascontiguousarray(np.asarray(router_weights, dtype=np.float32))
    in_maps = []
    for i in range(N_CORES):
        in_maps.append(
            {
                "x": xf[i * T_LOC : (i + 1) * T_LOC],
                "w": wf,
            }
        )
    return in_maps


def _assemble(results):
    idx = np.concatenate([r["indices"] for r in results], axis=0).reshape(B, S, TOPK)
    gat = np.concatenate([r["gates"] for r in results], axis=0).reshape(B, S, TOPK)
    lgt = np.concatenate([r["logits"] for r in results], axis=0).reshape(B, S, E)
    return idx.astype(np.int32), gat.astype(np.float32), lgt.astype(np.float32)


def kernel(x: np.ndarray, router_weights: np.ndarray):
    nc = _get_nc()
    in_maps = _shard_inputs(x, router_weights)
    res = run_bass_kernel_spmd(nc, in_maps, core_ids=list(range(N_CORES)))
    return _assemble(res.results)


def kernel_traced(x: np.ndarray, router_weights: np.ndarray, trace_cores=None):
    """Like kernel() but profiles; returns (outputs, BassKernelResults)."""
    nc = _get_nc()
    in_maps = _shard_inputs(x, router_weights)
    res = run_bass_kernel_spmd(
        nc,
        in_maps,
        core_ids=list(range(N_CORES)),
        trace=True,
        trace_cores=trace_cores or [0],
    )
    return _assemble(res.results), res


# revision 9
# speedup vs baseline: 1.5362x; 1.0361x over previous
"""MoE router kernel for Trainium2 (Bass/Tile), SPMD over 8 NeuronCores.

Reference computation (full problem):
    logits = einsum('bsd,ed->bse', x, W)     x: [4, 4096, 2048] f32, W: [8, 2048] f32
    top_vals, top_idx = top_k(logits, 2)
    gates = softmax(top_vals, axis=-1)
    returns (top_idx int32, gates f32, logits f32)

Sharding: data parallel over tokens (batch*seq = 16384) -> 2048 tokens/core,
router weights replicated. Each core:
  - token t = 16*p + b  (p = SBUF partition 0..127, b = token block 0..15), so
    every DMA is contiguous per partition (x rows: 8KB, outputs: 128-512B).
  - Per super-block of 512 tokens: PE-transpose x 128x128 chunks into PSUM,
    copy to SBUF (ACT/DVE alternating), then router matmul with W^T chunks
    stationary ([128,8]) and x^T moving (N=512), accumulating logits^T [8,512]
    in PSUM over the 16 contraction chunks.
  - PE-transpose logits^T -> [128 tok, 8]; DVE max/max_index give the top-8
    sorted values + indices per token (exactly top-k for E=8); batched
    exp/sum/reciprocal softmax over the top-2.
"""

import os
import sys

for _p in ("/opt/trn_rl_repo", "/root/.axon_site", "/root/.axon_site/_ro/trn_rl_repo",
           "/root/.axon_site/_ro/pypackages"):
    if os.path.isdir(_p) and _p not in sys.path:
        sys.path.append(_p)

import numpy as np

import concourse.bass as bass  # noqa: F401
import concourse.mybir as mybir
from concourse import bacc, tile
from concourse.bass_utils import run_bass_kernel_spmd
from concourse.masks import make_identity

# ---- problem constants (hardcoded; kernel.py must be self-contained) ----
B, S, D, E, TOPK = 4, 4096, 2048, 8, 2
N_CORES = 8
T_ALL = B * S                 # 16384 tokens
T_LOC = T_ALL // N_CORES      # 2048 tokens per core
P = 128                       # SBUF partitions
NB = T_LOC // P               # 16 token blocks per core
SBK = 4                       # blocks per super-block (512 tokens)
NSB = NB // SBK               # 4 super-blocks
DC = D // P                   # 16 contraction chunks

f32 = mybir.dt.float32
f32r = mybir.dt.float32r
i32 = mybir.dt.int32
u32 = mybir.dt.uint32

def build_nc():
    nc = bacc.Bacc(
        "TRN2",
        target_bir_lowering=False,
        debug=False,
        enable_asserts=False,
        num_devices=N_CORES,
    )
    x = nc.dram_tensor("x", [T_LOC, D], f32, kind="ExternalInput").ap()
    w = nc.dram_tensor("w", [E, D], f32, kind="ExternalInput").ap()
    idx_d = nc.dram_tensor("indices", [T_LOC, TOPK], i32, kind="ExternalOutput").ap()
    gat_d = nc.dram_tensor("gates", [T_LOC, TOPK], f32, kind="ExternalOutput").ap()
    log_d = nc.dram_tensor("logits", [T_LOC, E], f32, kind="ExternalOutput").ap()

    # token t = 16*p + b
    x_v = x.rearrange("(p b) d -> b p d", b=NB)           # [16, 128, 2048]
    idx_v = idx_d.rearrange("(p b) k -> p (b k)", b=NB)   # [128, 32]
    gat_v = gat_d.rearrange("(p b) k -> p (b k)", b=NB)   # [128, 32]
    log_v = log_d.rearrange("(p b) e -> p (b e)", b=NB)   # [128, 128]

    with tile.TileContext(nc) as tc:
        with (
            tc.tile_pool(name="const", bufs=1) as cpool,
            tc.tile_pool(name="xin", bufs=2 * SBK) as xpool,
            tc.tile_pool(name="xt", bufs=4) as xtpool,
            tc.tile_pool(name="acc", bufs=1) as apool,
            tc.tile_pool(name="ps_xt", bufs=3, space="PSUM") as pxt,
            tc.tile_pool(name="ps_out", bufs=2, space="PSUM") as pout,
            tc.tile_pool(name="ps_tr", bufs=2, space="PSUM") as ptr,
        ):
            # ---------------- constants ----------------
            ident = cpool.tile([P, P], f32)
            make_identity(nc, ident[:])
            w_sb = cpool.tile([E, D], f32)
            nc.sync.dma_start(out=w_sb[:], in_=w)
            # W^T chunks: wt_sb[:, 8c:8c+8] = W[:, 128c:128c+128].T  ([128 d, 8 e])
            wt_sb = cpool.tile([P, DC * E], f32)
            for h in range(2):
                wt_ps = ptr.tile([P, P], f32, tag="tr")
                for ci in range(8):
                    c = 8 * h + ci
                    nc.tensor.transpose(
                        wt_ps[:, E * ci : E * (ci + 1)],
                        w_sb[:, P * c : P * (c + 1)],
                        ident[0:E, 0:E],
                    )
                nc.vector.tensor_copy(wt_sb[:, 64 * h : 64 * (h + 1)], wt_ps[:, 0:64])

            # tfloat32 split of W^T: wt_r = R(W^T), wt_res = R(W^T - R(W^T)).
            # The router matmul runs three fp32r passes (R(x)Rw + R(rx)Rw +
            # R(x)Rrw), recovering fp32-level precision at 1 cycle/row.
            wt_r = cpool.tile([P, DC * E], f32r)
            wt_res = cpool.tile([P, DC * E], f32r)
            nc.scalar.copy(wt_r[:], wt_sb[:])
            nc.vector.tensor_sub(wt_res[:], wt_sb[:], wt_r[:])

            # HAM warmup: ~4.5us of back-to-back matmuls so the PE clock-gate
            # opens to 8/8 before the transpose/matmul stream begins.
            warm_ps = ptr.tile([P, P], f32, tag="tr")
            for _ in range(22):
                nc.tensor.matmul(warm_ps[:], ident[:], ident[:], start=True, stop=True)

            # ---------------- per-core accumulators ----------------
            out_log3 = apool.tile([P, NB, E], f32)    # logits, token-major layout
            tops3 = apool.tile([P, NB, E], f32)       # top-8 sorted values
            idx3 = apool.tile([P, NB, E], u32)        # their indices
            out_idx3 = apool.tile([P, NB, TOPK], i32)
            out_gat3 = apool.tile([P, NB, TOPK], f32)
            ex_in = apool.tile([P, NB, TOPK], f32)
            ex = apool.tile([P, NB, TOPK], f32)
            ssum = apool.tile([P, NB, 1], f32)
            rcp = apool.tile([P, NB, 1], f32)

            # ---------------- main loop over super-blocks ----------------
            for s in range(NSB):
                xbs = []
                for j in range(SBK):
                    b = SBK * s + j
                    xb = xpool.tile([P, D], f32, tag="xb")
                    nc.sync.dma_start(out=xb[:], in_=x_v[b])
                    xbs.append(xb)

                # logits^T [8, 512] accumulated over the 16 contraction chunks
                logT_ps = pout.tile([E, 4 * P], f32, tag="lgT")
                for c in range(DC):
                    # x^T chunk c for all 4 blocks: xt[:, 128j + q] = x[tok q of
                    # block 4s+j, 128c + p]
                    xt_ps = pxt.tile([P, 4 * P], f32, tag="xt_ps")
                    for j in range(SBK):
                        nc.tensor.transpose(
                            xt_ps[:, P * j : P * (j + 1)],
                            xbs[j][:, P * c : P * (c + 1)],
                            ident[:],
                        )
                    # tfloat32 split of x^T chunk: xt_r = R(x^T) (ACT cast),
                    # xt_res = R(x^T - R(x^T)) (DVE)
                    xt_r = xtpool.tile([P, 4 * P], f32r, tag="xt_r")
                    xt_res = xtpool.tile([P, 4 * P], f32r, tag="xt_res")
                    nc.scalar.copy(xt_r[:], xt_ps[:])
                    nc.vector.tensor_sub(xt_res[:], xt_ps[:], xt_r[:])
                    # three fp32r passes, all accumulating into logT_ps
                    nc.tensor.matmul(
                        logT_ps[:],
                        wt_r[:, E * c : E * (c + 1)],
                        xt_r[:],
                        start=(c == 0),
                        stop=False,
                    )
                    nc.tensor.matmul(
                        logT_ps[:],
                        wt_r[:, E * c : E * (c + 1)],
                        xt_res[:],
                        start=False,
                        stop=False,
                    )
                    nc.tensor.matmul(
                        logT_ps[:],
                        wt_res[:, E * c : E * (c + 1)],
                        xt_r[:],
                        start=False,
                        stop=(c == DC - 1),
                    )

                logT_sb = xtpool.tile([E, 4 * P], f32, tag="lgTsb")
                nc.scalar.copy(logT_sb[:], logT_ps[:])

                # transpose logits^T -> [128 tok, 8] per block; top-k
                for j in range(SBK):
                    b = SBK * s + j
                    ltr_ps = ptr.tile([P, E], f32, tag="tr")
                    nc.tensor.transpose(
                        ltr_ps[:], logT_sb[:, P * j : P * (j + 1)], ident[0:E, 0:E]
                    )
                    nc.vector.tensor_copy(out_log3[:, b, :], ltr_ps[:])
                    nc.vector.max(out=tops3[:, b, :], in_=out_log3[:, b, :])
                    nc.vector.max_index(
                        out=idx3[:, b, :],
                        in_max=tops3[:, b, :],
                        in_values=out_log3[:, b, :],
                    )

            # ---------------- batched softmax over top-2 ----------------
            nc.vector.tensor_sub(
                ex_in[:],
                tops3[:, :, 0:TOPK],
                tops3[:, :, 0:1].to_broadcast([P, NB, TOPK]),
            )
            nc.scalar.activation(ex[:], ex_in[:], mybir.ActivationFunctionType.Exp)
            nc.vector.tensor_reduce(
                ssum[:], ex[:], axis=mybir.AxisListType.X, op=mybir.AluOpType.add
            )
            nc.vector.reciprocal(rcp[:], ssum[:])
            nc.vector.tensor_mul(
                out_gat3[:], ex[:], rcp.to_broadcast([P, NB, TOPK])
            )
            nc.vector.tensor_copy(out_idx3[:], idx3[:, :, 0:TOPK])

            # ---------------- outputs ----------------
            nc.sync.dma_start(out=idx_v, in_=out_idx3[:])
            nc.sync.dma_start(out=gat_v, in_=out_gat3[:])
            nc.sync.dma_start(out=log_v, in_=out_log3[:])

    nc.compile()
    return nc


_NC_CACHE = None


def _get_nc():
    global _NC_CACHE
    if _NC_CACHE is None:
        _NC_CACHE = build_nc()
    return _NC_CACHE


def _shard_inputs(x: np.ndarray, router_weights: np.ndarray):
    xf = np.ascontiguousarray(np.asarray(x, dtype=np.float32).reshape(T_ALL, D))
    wf = np.ascontiguousarray(np.asarray(router_weights, dtype=np.float32))
    in_maps = []
    for i in range(N_CORES):
        in_maps.append(
            {
                "x": xf[i * T_LOC : (i + 1) * T_LOC],
                "w": wf,
            }
        )
    return in_maps


def _assemble(results):
    idx = np.concatenate([r["indices"] for r in results], axis=0).reshape(B, S, TOPK)
    gat = np.concatenate([r["gates"] for r in results], axis=0).reshape(B, S, TOPK)
    lgt = np.concatenate([r["logits"] for r in results], axis=0).reshape(B, S, E)
    return idx.astype(np.int32), gat.astype(np.float32), lgt.astype(np.float32)


def kernel(x: np.ndarray, router_weights: np.ndarray):
    nc = _get_nc()
    in_maps = _shard_inputs(x, router_weights)
    res = run_bass_kernel_spmd(nc, in_maps, core_ids=list(range(N_CORES)))
    return _assemble(res.results)


def kernel_traced(x: np.ndarray, router_weights: np.ndarray, trace_cores=None):
    """Like kernel() but profiles; returns (outputs, BassKernelResults)."""
    nc = _get_nc()
    in_maps = _shard_inputs(x, router_weights)
    res = run_bass_kernel_spmd(
        nc,
        in_maps,
        core_ids=list(range(N_CORES)),
        trace=True,
        trace_cores=trace_cores or [0],
    )
    return _assemble(res.results), res


# revision 19
# speedup vs baseline: 1.9509x; 1.2700x over previous
"""MoE router kernel for Trainium2 (Bass/Tile), SPMD over 8 NeuronCores.

Reference computation (full problem):
    logits = einsum('bsd,ed->bse', x, W)     x: [4, 4096, 2048] f32, W: [8, 2048] f32
    top_vals, top_idx = top_k(logits, 2)
    gates = softmax(top_vals, axis=-1)
    returns (top_idx int32, gates f32, logits f32)

Sharding: data parallel over tokens (batch*seq = 16384) -> 2048 tokens/core,
router weights replicated. Each core:
  - token t = 16*p + b  (p = SBUF partition 0..127, b = token block 0..15), so
    every DMA is contiguous per partition (x rows: 8KB, outputs: 128-512B).
  - Per super-block of 512 tokens: PE-transpose x 128x128 chunks into PSUM,
    copy to SBUF (ACT/DVE alternating), then router matmul with W^T chunks
    stationary ([128,8]) and x^T moving (N=512), accumulating logits^T [8,512]
    in PSUM over the 16 contraction chunks.
  - PE-transpose logits^T -> [128 tok, 8]; DVE max/max_index give the top-8
    sorted values + indices per token (exactly top-k for E=8); batched
    exp/sum/reciprocal softmax over the top-2.
"""

import os
import sys

for _p in ("/opt/trn_rl_repo", "/root/.axon_site", "/root/.axon_site/_ro/trn_rl_repo",
           "/root/.axon_site/_ro/pypackages"):
    if os.path.isdir(_p) and _p not in sys.path:
        sys.path.append(_p)

import numpy as np

import concourse.bass as bass  # noqa: F401
import concourse.mybir as mybir
from concourse import bacc, tile
from concourse.bass_utils import run_bass_kernel_spmd
from concourse.masks import make_identity

# ---- problem constants (hardcoded; kernel.py must be self-contained) ----
B, S, D, E, TOPK = 4, 4096, 2048, 8, 2
N_CORES = 8
T_ALL = B * S                 # 16384 tokens
T_LOC = T_ALL // N_CORES      # 2048 tokens per core
P = 128                       # SBUF partitions
NB = T_LOC // P               # 16 token blocks per core
SBK = 4                       # blocks per super-block (512 tokens)
NSB = NB // SBK               # 4 super-blocks
DC = D // P                   # 16 contraction chunks

f32 = mybir.dt.float32
f32r = mybir.dt.float32r
i32 = mybir.dt.int32
u32 = mybir.dt.uint32

def build_nc():
    nc = bacc.Bacc(
        "TRN2",
        target_bir_lowering=False,
        debug=False,
        enable_asserts=False,
        num_devices=N_CORES,
    )
    x = nc.dram_tensor("x", [T_LOC, D], f32, kind="ExternalInput").ap()
    w = nc.dram_tensor("w", [E, D], f32, kind="ExternalInput").ap()
    idx_d = nc.dram_tensor("indices", [T_LOC, TOPK], i32, kind="ExternalOutput").ap()
    gat_d = nc.dram_tensor("gates", [T_LOC, TOPK], f32, kind="ExternalOutput").ap()
    log_d = nc.dram_tensor("logits", [T_LOC, E], f32, kind="ExternalOutput").ap()

    # token t = 16*p + b
    x_v = x.rearrange("(p b) d -> b p d", b=NB)           # [16, 128, 2048]
    idx_v = idx_d.rearrange("(p b) k -> p (b k)", b=NB)   # [128, 32]
    gat_v = gat_d.rearrange("(p b) k -> p (b k)", b=NB)   # [128, 32]
    log_v = log_d.rearrange("(p b) e -> p (b e)", b=NB)   # [128, 128]

    with tile.TileContext(nc) as tc:
        with (
            tc.tile_pool(name="const", bufs=1) as cpool,
            tc.tile_pool(name="xin", bufs=2 * SBK) as xpool,
            tc.tile_pool(name="xt", bufs=4) as xtpool,
            tc.tile_pool(name="acc", bufs=1) as apool,
            tc.tile_pool(name="ps_xt", bufs=4, space="PSUM") as pxt,
            tc.tile_pool(name="ps_out", bufs=2, space="PSUM") as pout,
            tc.tile_pool(name="ps_tr", bufs=2, space="PSUM") as ptr,
        ):
            # ---------------- constants ----------------
            ident = cpool.tile([P, P], f32)
            make_identity(nc, ident[:])
            w_sb = cpool.tile([E, D], f32)
            nc.sync.dma_start(out=w_sb[:], in_=w)
            # W^T chunks: wt_sb[:, 8c:8c+8] = W[:, 128c:128c+128].T  ([128 d, 8 e])
            wt_sb = cpool.tile([P, DC * E], f32)
            for h in range(2):
                wt_ps = ptr.tile([P, P], f32, tag="tr")
                for ci in range(8):
                    c = 8 * h + ci
                    nc.tensor.transpose(
                        wt_ps[:, E * ci : E * (ci + 1)],
                        w_sb[:, P * c : P * (c + 1)],
                        ident[0:E, 0:E],
                    )
                nc.vector.tensor_copy(wt_sb[:, 64 * h : 64 * (h + 1)], wt_ps[:, 0:64])

            # tfloat32 split of W^T: wt_r = R(W^T), wt_res = R(W^T - R(W^T)).
            # Router runs as fp32r with residual correction: logits =
            # Rw.x_r + Rrw.x_r + Rw.x_res — fp32-level precision at 1 cyc/row.
            # wtcat packs [Rw_c | Rrw_c] per chunk into one [128, 16]
            # stationary so the first two terms share one moving pass, landing
            # in psum partitions 0:8 and 8:16.
            # WCAT = 40 columns per chunk: [Rw_c | zeros(24) | Rrw_c] — the
            # zero pad puts the Rrw product at psum partitions 32:40 (engine
            # reads need base partition in {0,32,64,96}).
            WCAT = 40
            wt_r = cpool.tile([P, DC * E], f32r)
            wtcat = cpool.tile([P, DC * WCAT], f32r)
            nc.scalar.copy(wt_r[:], wt_sb[:])
            nc.vector.memset(wtcat[:].bitcast(f32), 0.0)
            wtcat3 = wtcat.rearrange("p (c w) -> p c w", w=WCAT)
            nc.vector.tensor_copy(
                wtcat3[:, :, 0:E], wt_r.rearrange("p (c e) -> p c e", e=E)
            )
            nc.vector.tensor_sub(
                wtcat3[:, :, 32 : 32 + E],
                wt_sb.rearrange("p (c e) -> p c e", e=E),
                wt_r.rearrange("p (c e) -> p c e", e=E),
            )

            # HAM warmup: ~4.5us of back-to-back matmuls so the PE clock-gate
            # opens to 8/8 before the transpose/matmul stream begins.
            warm_ps = ptr.tile([P, P], f32, tag="tr")
            for _ in range(22):
                nc.tensor.matmul(warm_ps[:], ident[:], ident[:], start=True, stop=True)

            # ---------------- per-core accumulators ----------------
            out_log3 = apool.tile([P, NB, E], f32)    # logits, token-major layout
            tops3 = apool.tile([P, NB, E], f32)       # top-8 sorted values
            idx3 = apool.tile([P, NB, E], u32)        # their indices
            out_idx3 = apool.tile([P, NB, TOPK], i32)
            out_gat3 = apool.tile([P, NB, TOPK], f32)
            ex_in = apool.tile([P, NB, TOPK], f32)
            ex = apool.tile([P, NB, TOPK], f32)
            ssum = apool.tile([P, NB, 1], f32)
            rcp = apool.tile([P, NB, 1], f32)

            # ---------------- main loop over super-blocks ----------------
            for s in range(NSB):
                xbs = []
                for j in range(SBK):
                    b = SBK * s + j
                    xb = xpool.tile([P, D], f32, tag="xb")
                    nc.sync.dma_start(out=xb[:], in_=x_v[b])
                    xbs.append(xb)

                # logits^T accumulator: rows 0:8 = Rw.x_r + Rw.x_res,
                # rows 32:40 = Rrw.x_r; summed after the chunk loop
                logT_ps = pout.tile([WCAT, 4 * P], f32, tag="lgT")
                for c in range(DC):
                    # x^T chunk c for all 4 blocks: xt[:, 128j + q] = x[tok q of
                    # block 4s+j, 128c + p]
                    xt_ps = pxt.tile([P, 4 * P], f32, tag="xt_ps")
                    for j in range(SBK):
                        nc.tensor.transpose(
                            xt_ps[:, P * j : P * (j + 1)],
                            xbs[j][:, P * c : P * (c + 1)],
                            ident[:],
                        )
                    # tfloat32 split of x^T chunk: xt_r = R(x^T) (ACT cast),
                    # xt_res = R(x^T - R(x^T)) (DVE)
                    xt_r = xtpool.tile([P, 4 * P], f32r, tag="xt_r")
                    xt_res = xtpool.tile([P, 4 * P], f32r, tag="xt_res")
                    nc.scalar.copy(xt_r[:], xt_ps[:])
                    nc.vector.tensor_sub(xt_res[:], xt_ps[:], xt_r[:])
                    # two fp32r moving passes, accumulating into logT_ps.
                    # The group must open and close on the full-height (16-row)
                    # matmul, so the last chunk runs the residual pass first.
                    def mm_cat(stop):
                        nc.tensor.matmul(
                            logT_ps[:],
                            wtcat[:, WCAT * c : WCAT * (c + 1)],
                            xt_r[:],
                            start=(c == 0),
                            stop=stop,
                        )

                    def mm_res():
                        nc.tensor.matmul(
                            logT_ps[0:E, :],
                            wt_r[:, E * c : E * (c + 1)],
                            xt_res[:],
                            start=False,
                            stop=False,
                        )

                    if c < DC - 1:
                        mm_cat(False)
                        mm_res()
                    else:
                        mm_res()
                        mm_cat(True)

                logT_hi = xtpool.tile([E, 4 * P], f32, tag="lgThi")
                nc.scalar.copy(logT_hi[:], logT_ps[32 : 32 + E, :])
                logT_sb = xtpool.tile([E, 4 * P], f32, tag="lgTsb")
                nc.vector.tensor_add(logT_sb[:], logT_ps[0:E, :], logT_hi[:])

                # transpose logits^T -> [128 tok, 8] per block; top-k
                for j in range(SBK):
                    b = SBK * s + j
                    ltr_ps = ptr.tile([P, E], f32, tag="tr")
                    nc.tensor.transpose(
                        ltr_ps[:], logT_sb[:, P * j : P * (j + 1)], ident[0:E, 0:E]
                    )
                    nc.vector.tensor_copy(out_log3[:, b, :], ltr_ps[:])
                    nc.vector.max(out=tops3[:, b, :], in_=out_log3[:, b, :])
                    nc.vector.max_index(
                        out=idx3[:, b, :],
                        in_max=tops3[:, b, :],
                        in_values=out_log3[:, b, :],
                    )

            # ---------------- batched softmax over top-2 ----------------
            nc.vector.tensor_sub(
                ex_in[:],
                tops3[:, :, 0:TOPK],
                tops3[:, :, 0:1].to_broadcast([P, NB, TOPK]),
            )
            nc.scalar.activation(ex[:], ex_in[:], mybir.ActivationFunctionType.Exp)
            nc.vector.tensor_reduce(
                ssum[:], ex[:], axis=mybir.AxisListType.X, op=mybir.AluOpType.add
            )
            nc.vector.reciprocal(rcp[:], ssum[:])
            nc.vector.tensor_mul(
                out_gat3[:], ex[:], rcp.to_broadcast([P, NB, TOPK])
            )
            nc.vector.tensor_copy(out_idx3[:], idx3[:, :, 0:TOPK])

            # ---------------- outputs ----------------
            nc.sync.dma_start(out=idx_v, in_=out_idx3[:])
            nc.sync.dma_start(out=gat_v, in_=out_gat3[:])
            nc.sync.dma_start(out=log_v, in_=out_log3[:])

    nc.compile()
    return nc


_NC_CACHE = None


def _get_nc():
    global _NC_CACHE
    if _NC_CACHE is None:
        _NC_CACHE = build_nc()
    return _NC_CACHE


def _shard_inputs(x: np.ndarray, router_weights: np.ndarray):
    xf = np.ascontiguousarray(np.asarray(x, dtype=np.float32).reshape(T_ALL, D))
    wf = np.ascontiguousarray(np.asarray(router_weights, dtype=np.float32))
    in_maps = []
    for i in range(N_CORES):
        in_maps.append(
            {
                "x": xf[i * T_LOC : (i + 1) * T_LOC],
                "w": wf,
            }
        )
    return in_maps


def _assemble(results):
    idx = np.concatenate([r["indices"] for r in results], axis=0).reshape(B, S, TOPK)
    gat = np.concatenate([r["gates"] for r in results], axis=0).reshape(B, S, TOPK)
    lgt = np.concatenate([r["logits"] for r in results], axis=0).reshape(B, S, E)
    return idx.astype(np.int32), gat.astype(np.float32), lgt.astype(np.float32)


def kernel(x: np.ndarray, router_weights: np.ndarray):
    nc = _get_nc()
    in_maps = _shard_inputs(x, router_weights)
    res = run_bass_kernel_spmd(nc, in_maps, core_ids=list(range(N_CORES)))
    return _assemble(res.results)


def kernel_traced(x: np.ndarray, router_weights: np.ndarray, trace_cores=None):
    """Like kernel() but profiles; returns (outputs, BassKernelResults)."""
    nc = _get_nc()
    in_maps = _shard_inputs(x, router_weights)
    res = run_bass_kernel_spmd(
        nc,
        in_maps,
        core_ids=list(range(N_CORES)),
        trace=True,
        trace_cores=trace_cores or [0],
    )
    return _assemble(res.results), res


# revision 20
# speedup vs baseline: 2.1001x; 1.0765x over previous
"""MoE router kernel for Trainium2 (Bass/Tile), SPMD over 8 NeuronCores.

Reference computation (full problem):
    logits = einsum('bsd,ed->bse', x, W)     x: [4, 4096, 2048] f32, W: [8, 2048] f32
    top_vals, top_idx = top_k(logits, 2)
    gates = softmax(top_vals, axis=-1)
    returns (top_idx int32, gates f32, logits f32)

Sharding: data parallel over tokens (batch*seq = 16384) -> 2048 tokens/core,
router weights replicated. Each core:
  - token t = 16*p + b  (p = SBUF partition 0..127, b = token block 0..15), so
    every DMA is contiguous per partition (x rows: 8KB, outputs: 128-512B).
  - Per super-block of 512 tokens: PE-transpose x 128x128 chunks into PSUM,
    copy to SBUF (ACT/DVE alternating), then router matmul with W^T chunks
    stationary ([128,8]) and x^T moving (N=512), accumulating logits^T [8,512]
    in PSUM over the 16 contraction chunks.
  - PE-transpose logits^T -> [128 tok, 8]; DVE max/max_index give the top-8
    sorted values + indices per token (exactly top-k for E=8); batched
    exp/sum/reciprocal softmax over the top-2.
"""

import os
import sys

for _p in ("/opt/trn_rl_repo", "/root/.axon_site", "/root/.axon_site/_ro/trn_rl_repo",
           "/root/.axon_site/_ro/pypackages"):
    if os.path.isdir(_p) and _p not in sys.path:
        sys.path.append(_p)

import numpy as np

import concourse.bass as bass  # noqa: F401
import concourse.mybir as mybir
from concourse import bacc, tile
from concourse.bass_utils import run_bass_kernel_spmd
from concourse.masks import make_identity

# ---- problem constants (hardcoded; kernel.py must be self-contained) ----
B, S, D, E, TOPK = 4, 4096, 2048, 8, 2
N_CORES = 8
T_ALL = B * S                 # 16384 tokens
T_LOC = T_ALL // N_CORES      # 2048 tokens per core
P = 128                       # SBUF partitions
NB = T_LOC // P               # 16 token blocks per core
SBK = 4                       # blocks per super-block (512 tokens)
NSB = NB // SBK               # 4 super-blocks
DC = D // P                   # 16 contraction chunks

f32 = mybir.dt.float32
f32r = mybir.dt.float32r
i32 = mybir.dt.int32
u32 = mybir.dt.uint32

def build_nc():
    nc = bacc.Bacc(
        "TRN2",
        target_bir_lowering=False,
        debug=False,
        enable_asserts=False,
        num_devices=N_CORES,
    )
    x = nc.dram_tensor("x", [T_LOC, D], f32, kind="ExternalInput").ap()
    w = nc.dram_tensor("w", [E, D], f32, kind="ExternalInput").ap()
    idx_d = nc.dram_tensor("indices", [T_LOC, TOPK], i32, kind="ExternalOutput").ap()
    gat_d = nc.dram_tensor("gates", [T_LOC, TOPK], f32, kind="ExternalOutput").ap()
    log_d = nc.dram_tensor("logits", [T_LOC, E], f32, kind="ExternalOutput").ap()

    # token t = 16*p + b
    x_v = x.rearrange("(p b) d -> b p d", b=NB)           # [16, 128, 2048]
    idx_v = idx_d.rearrange("(p b) k -> p (b k)", b=NB)   # [128, 32]
    gat_v = gat_d.rearrange("(p b) k -> p (b k)", b=NB)   # [128, 32]
    log_v = log_d.rearrange("(p b) e -> p (b e)", b=NB)   # [128, 128]

    with tile.TileContext(nc) as tc:
        with (
            tc.tile_pool(name="const", bufs=1) as cpool,
            tc.tile_pool(name="xin", bufs=2 * SBK) as xpool,
            tc.tile_pool(name="xt", bufs=4) as xtpool,
            tc.tile_pool(name="acc", bufs=1) as apool,
            tc.tile_pool(name="ps_xt", bufs=4, space="PSUM") as pxt,
            tc.tile_pool(name="ps_out", bufs=2, space="PSUM") as pout,
            tc.tile_pool(name="ps_tr", bufs=2, space="PSUM") as ptr,
        ):
            # ---------------- constants ----------------
            ident = cpool.tile([P, P], f32)
            make_identity(nc, ident[:])
            w_sb = cpool.tile([E, D], f32)
            nc.sync.dma_start(out=w_sb[:], in_=w)
            # W^T chunks: wt_sb[:, 8c:8c+8] = W[:, 128c:128c+128].T  ([128 d, 8 e])
            wt_sb = cpool.tile([P, DC * E], f32)
            for h in range(2):
                wt_ps = ptr.tile([P, P], f32, tag="tr")
                for ci in range(8):
                    c = 8 * h + ci
                    nc.tensor.transpose(
                        wt_ps[:, E * ci : E * (ci + 1)],
                        w_sb[:, P * c : P * (c + 1)],
                        ident[0:E, 0:E],
                    )
                nc.vector.tensor_copy(wt_sb[:, 64 * h : 64 * (h + 1)], wt_ps[:, 0:64])

            # tfloat32 split of W^T: wt_r = R(W^T), wt_res = R(W^T - R(W^T)).
            # Router runs as fp32r with residual correction: logits =
            # Rw.x_r + Rrw.x_r + Rw.x_res — fp32-level precision at 1 cyc/row.
            # wtcat packs [Rw_c | Rrw_c] per chunk into one [128, 16]
            # stationary so the first two terms share one moving pass, landing
            # in psum partitions 0:8 and 8:16.
            # WCAT = 40 columns per chunk: [Rw_c | zeros(24) | Rrw_c] — the
            # zero pad puts the Rrw product at psum partitions 32:40 (engine
            # reads need base partition in {0,32,64,96}).
            WCAT = 40
            wt_r = cpool.tile([P, DC * E], f32r)
            wtcat = cpool.tile([P, DC * WCAT], f32r)
            nc.scalar.copy(wt_r[:], wt_sb[:])
            nc.vector.memset(wtcat[:].bitcast(f32), 0.0)
            wtcat3 = wtcat.rearrange("p (c w) -> p c w", w=WCAT)
            nc.vector.tensor_copy(
                wtcat3[:, :, 0:E], wt_r.rearrange("p (c e) -> p c e", e=E)
            )
            nc.vector.tensor_sub(
                wtcat3[:, :, 32 : 32 + E],
                wt_sb.rearrange("p (c e) -> p c e", e=E),
                wt_r.rearrange("p (c e) -> p c e", e=E),
            )

            # f32r identity for the f32r transposes (ACT cast = the
            # verifier-recognized rounding producer; 0/1 round exactly)
            ident_r = cpool.tile([P, P], f32r)
            nc.scalar.copy(ident_r[:], ident[:])

            # HAM warmup: ~4.5us of back-to-back matmuls so the PE clock-gate
            # opens to 8/8 before the transpose/matmul stream begins.
            warm_ps = ptr.tile([P, P], f32, tag="tr")
            for _ in range(22):
                nc.tensor.matmul(warm_ps[:], ident[:], ident[:], start=True, stop=True)

            # ---------------- per-core accumulators ----------------
            out_log3 = apool.tile([P, NB, E], f32)    # logits, token-major layout
            tops3 = apool.tile([P, NB, E], f32)       # top-8 sorted values
            idx3 = apool.tile([P, NB, E], u32)        # their indices
            out_idx3 = apool.tile([P, NB, TOPK], i32)
            out_gat3 = apool.tile([P, NB, TOPK], f32)
            ex_in = apool.tile([P, NB, TOPK], f32)
            ex = apool.tile([P, NB, TOPK], f32)
            ssum = apool.tile([P, NB, 1], f32)
            rcp = apool.tile([P, NB, 1], f32)

            # ---------------- main loop over super-blocks ----------------
            for s in range(NSB):
                xbs = []
                for j in range(SBK):
                    b = SBK * s + j
                    xb = xpool.tile([P, D], f32r, tag="xb")
                    nc.sync.dma_start(out=xb[:], in_=x_v[b].bitcast(f32r))
                    xbs.append(xb)

                # logits^T accumulator: rows 0:8 = Rw.x_r + Rw.x_res,
                # rows 32:40 = Rrw.x_r; summed after the chunk loop
                logT_ps = pout.tile([WCAT, 4 * P], f32, tag="lgT")
                for c in range(DC):
                    # x^T chunk c for all 4 blocks: xt[:, 128j + q] = x[tok q of
                    # block 4s+j, 128c + p]
                    xt_ps = pxt.tile([P, 4 * P], f32r, tag="xt_ps")
                    for j in range(SBK):
                        nc.tensor.transpose(
                            xt_ps[:, P * j : P * (j + 1)],
                            xbs[j][:, P * c : P * (c + 1)],
                            ident_r[:],
                        )
                    # tfloat32 split of x^T chunk: xt_r = R(x^T) (ACT cast),
                    # xt_res = R(x^T - R(x^T)) (DVE)
                    xt_r = xtpool.tile([P, 4 * P], f32r, tag="xt_r")
                    xt_res = xtpool.tile([P, 4 * P], f32r, tag="xt_res")
                    nc.scalar.copy(xt_r[:], xt_ps[:])
                    nc.vector.tensor_sub(xt_res[:], xt_ps[:], xt_r[:])
                    # two fp32r moving passes, accumulating into logT_ps.
                    # The group must open and close on the full-height (16-row)
                    # matmul, so the last chunk runs the residual pass first.
                    def mm_cat(stop):
                        nc.tensor.matmul(
                            logT_ps[:],
                            wtcat[:, WCAT * c : WCAT * (c + 1)],
                            xt_r[:],
                            start=(c == 0),
                            stop=stop,
                        )

                    def mm_res():
                        nc.tensor.matmul(
                            logT_ps[0:E, :],
                            wt_r[:, E * c : E * (c + 1)],
                            xt_res[:],
                            start=False,
                            stop=False,
                        )

                    if c < DC - 1:
                        mm_cat(False)
                        mm_res()
                    else:
                        mm_res()
                        mm_cat(True)

                logT_hi = xtpool.tile([E, 4 * P], f32, tag="lgThi")
                nc.scalar.copy(logT_hi[:], logT_ps[32 : 32 + E, :])
                logT_sb = xtpool.tile([E, 4 * P], f32, tag="lgTsb")
                nc.vector.tensor_add(logT_sb[:], logT_ps[0:E, :], logT_hi[:])

                # transpose logits^T -> [128 tok, 8] per block; top-k
                for j in range(SBK):
                    b = SBK * s + j
                    ltr_ps = ptr.tile([P, E], f32, tag="tr")
                    nc.tensor.transpose(
                        ltr_ps[:], logT_sb[:, P * j : P * (j + 1)], ident[0:E, 0:E]
                    )
                    nc.vector.tensor_copy(out_log3[:, b, :], ltr_ps[:])
                    nc.vector.max(out=tops3[:, b, :], in_=out_log3[:, b, :])
                    nc.vector.max_index(
                        out=idx3[:, b, :],
                        in_max=tops3[:, b, :],
                        in_values=out_log3[:, b, :],
                    )

            # ---------------- batched softmax over top-2 ----------------
            nc.vector.tensor_sub(
                ex_in[:],
                tops3[:, :, 0:TOPK],
                tops3[:, :, 0:1].to_broadcast([P, NB, TOPK]),
            )
            nc.scalar.activation(ex[:], ex_in[:], mybir.ActivationFunctionType.Exp)
            nc.vector.tensor_reduce(
                ssum[:], ex[:], axis=mybir.AxisListType.X, op=mybir.AluOpType.add
            )
            nc.vector.reciprocal(rcp[:], ssum[:])
            nc.vector.tensor_mul(
                out_gat3[:], ex[:], rcp.to_broadcast([P, NB, TOPK])
            )
            nc.vector.tensor_copy(out_idx3[:], idx3[:, :, 0:TOPK])

            # ---------------- outputs ----------------
            nc.sync.dma_start(out=idx_v, in_=out_idx3[:])
            nc.sync.dma_start(out=gat_v, in_=out_gat3[:])
            nc.sync.dma_start(out=log_v, in_=out_log3[:])

    nc.compile()
    return nc


_NC_CACHE = None


def _get_nc():
    global _NC_CACHE
    if _NC_CACHE is None:
        _NC_CACHE = build_nc()
    return _NC_CACHE


def _shard_inputs(x: np.ndarray, router_weights: np.ndarray):
    xf = np.ascontiguousarray(np.asarray(x, dtype=np.float32).reshape(T_ALL, D))
    wf = np.ascontiguousarray(np.asarray(router_weights, dtype=np.float32))
    in_maps = []
    for i in range(N_CORES):
        in_maps.append(
            {
                "x": xf[i * T_LOC : (i + 1) * T_LOC],
                "w": wf,
            }
        )
    return in_maps


def _assemble(results):
    idx = np.concatenate([r["indices"] for r in results], axis=0).reshape(B, S, TOPK)
    gat = np.concatenate([r["gates"] for r in results], axis=0).reshape(B, S, TOPK)
    lgt = np.concatenate([r["logits"] for r in results], axis=0).reshape(B, S, E)
    return idx.astype(np.int32), gat.astype(np.float32), lgt.astype(np.float32)


def kernel(x: np.ndarray, router_weights: np.ndarray):
    nc = _get_nc()
    in_maps = _shard_inputs(x, router_weights)
    res = run_bass_kernel_spmd(nc, in_maps, core_ids=list(range(N_CORES)))
    return _assemble(res.results)


def kernel_traced(x: np.ndarray, router_weights: np.ndarray, trace_cores=None):
    """Like kernel() but profiles; returns (outputs, BassKernelResults)."""
    nc = _get_nc()
    in_maps = _shard_inputs(x, router_weights)
    res = run_bass_kernel_spmd(
        nc,
        in_maps,
        core_ids=list(range(N_CORES)),
        trace=True,
        trace_cores=trace_cores or [0],
    )
    return _assemble(res.results), res
